# revision 7
# baseline (speedup 1.0000x reference)
"""DAHH hypergraph conv (gnn_message_passing) on 8 Trainium2 NeuronCores.

reference:
    xp      = x @ theta                      [N, 64]
    de      = colsum(H)                      [E]
    edge_ft = (H.T @ xp) / de[:, None]       [E, 64]
    dn      = rowsum(H)                      [N]
    node_ft = (H @ edge_ft) / dn[:, None]    [N, 64]

Sharding: H and x row-sharded (node dim) across 8 cores; theta replicated.

Single-read pipelined design (v2): H is read from HBM ONCE per core
(100 MB f32). The edge dim is split into NCH chunks; per chunk c:
  - 20 node-tile DMAs land H[k, chunk] f32, cast to bf16 (DVE/ACT split)
  - phase-1 matmuls: psum[e,65] += H_tile.T @ [xp|1]  (H bf16 as PE weights,
    FWL; edges directly on psum partitions -- no transposes needed)
  - the same bf16 tile is block-transposed by the DMA XBAR (one
    dma_start(transpose=True) per node tile, 3D out AP) into a persistent
    SBUF hT chunk buffer [e,n] -- no PE/DVE cost
  - chunk AllReduce (f32, 266 KB) over the 8 node shards via DRAM bounce
  - normalize -> eft1 chunk [e, 64|1] bf16
  - phase-4 matmuls: po[n, 65] += hT_block.T @ eft1  accumulated in packed
    psum slices across all chunks
Software-pipelined with depth D=2: phase-4 of chunk c is emitted after
phase-1 of chunk c+2, so the per-chunk AllReduce latency hides under two
chunks of phase-1 work. hT chunk buffers rotate 3-deep.

N padded 20000->20480 (2560/core), E padded 10000->10240 with zeros;
padding is numerically inert (zero rows/cols, degree clamps via
max(.,1e-20)).
"""
import sys
sys.path.insert(0, "/opt/trn_rl_repo")
import numpy as np

import concourse.bass as bass
import concourse.bacc as bacc
import concourse.tile as tile
import concourse.mybir as mybir
from concourse.bass_utils import run_bass_kernel_spmd

N, E, IN_CH, OUT_CH = 20000, 10000, 128, 64
N_CORES = 8
NL = 2560            # padded nodes per core
EP = 10240           # padded edges
NT = NL // 128       # 20 node tiles per core
ET = EP // 128       # 80 edge tiles
W = 65               # 64 features + degree/ones column

CCOLS = 1024         # edge cols per chunk
CET = CCOLS // 128   # 8 edge tiles per chunk
NCH = EP // CCOLS    # 10 chunks
D = 2                # software pipeline depth (phase4 lag in chunks)
HTBUFS = 3           # hT chunk buffers in flight

f32 = mybir.dt.float32
bf16 = mybir.dt.bfloat16

SKIP_COLLECTIVE = False   # dev-only: replace AllReduce with local copy
USE_XBAR = True           # False: PE transposes instead of DMA XBAR


def build_body(nc, tc, x_ext, H_ext, th_ext, id_ext, out_ext, sfx="",
               chain_in=None, chain_out=None):
    PACK = 7  # po accumulators packed per psum bank
    with (
        tc.tile_pool(name=f"const{sfx}", bufs=1) as constp,
        tc.tile_pool(name=f"persist{sfx}", bufs=1) as persist,
        tc.tile_pool(name=f"dram{sfx}", bufs=1, space="DRAM") as dram,
    ):
        ident = constp.tile([128, 128], f32)
        nc.sync.dma_start(ident[:], id_ext[:])
        identb = constp.tile([128, 128], bf16)
        nc.vector.tensor_copy(identb[:], ident[:])
        th_f = constp.tile([128, OUT_CH], f32)
        nc.sync.dma_start(th_f[:], th_ext[:])
        if chain_in is not None:
            # timing-only serialization: theta (hence everything downstream)
            # gains a data dependency on the previous rep's output
            chs = constp.tile([1, 1], f32)
            nc.sync.dma_start(chs[:], chain_in[:])
            chz = constp.tile([1, 1], f32)
            nc.vector.tensor_scalar_mul(chz[:], chs[:], 0.0)
            nc.vector.tensor_tensor(th_f[0:1, 0:1], th_f[0:1, 0:1],
                                    chz[:], mybir.AluOpType.add)
        th_b = constp.tile([128, OUT_CH], bf16)
        nc.vector.tensor_copy(th_b[:], th_f[:])

        xp1 = persist.tile([128, NT * W], bf16)

        # ---- phase 0: xp1 = [x @ theta | 1] per node tile ----
        with (
            tc.tile_pool(name=f"p0{sfx}", bufs=3) as p0,
            tc.tile_pool(name=f"p0ps{sfx}", bufs=2, space="PSUM") as p0ps,
        ):
            for k in range(NT):
                xt = p0.tile([128, 128], f32, tag="x")
                nc.sync.dma_start(xt[:], x_ext[k * 128:(k + 1) * 128, :])
                pt = p0ps.tile([128, 128], f32, tag="pt")
                nc.tensor.transpose(pt[:], xt[:], ident[:])
                xT = p0.tile([128, 128], bf16, tag="xT")
                nc.vector.tensor_copy(xT[:], pt[:])
                pxp = p0ps.tile([128, OUT_CH], f32, tag="pxp")
                nc.tensor.matmul(pxp[:], xT[:], th_b[:], start=True, stop=True)
                nc.vector.tensor_copy(xp1[:, k * W:k * W + OUT_CH], pxp[:])
                nc.vector.memset(xp1[:, k * W + OUT_CH:(k + 1) * W], 1.0)

        # ---- main pipelined loop over edge chunks ----
        bins, bouts = [], []
        for c in range(NCH):
            bins.append(dram.tile([128, CET * W], f32,
                                  name=f"bin{c}{sfx}", tag=f"bin{c}"))
            bouts.append(dram.tile([128, CET * W], f32, addr_space="Shared",
                                   name=f"bout{c}{sfx}", tag=f"bout{c}"))

        with (
            tc.tile_pool(name=f"hT{sfx}", bufs=HTBUFS) as hTp,
            tc.tile_pool(name=f"hf{sfx}", bufs=4) as hfp,
            tc.tile_pool(name=f"hb{sfx}", bufs=4) as hbp,
            tc.tile_pool(name=f"ar{sfx}", bufs=2) as arp,
            tc.tile_pool(name=f"ef{sfx}", bufs=2) as efp,
            tc.tile_pool(name=f"p1ps{sfx}", bufs=2, space="PSUM") as p1ps,
            tc.tile_pool(name=f"pops{sfx}", bufs=1, space="PSUM") as pops,
            tc.tile_pool(name=f"ptps{sfx}", bufs=2, space="PSUM") as ptps,
            tc.tile_pool(name=f"outp{sfx}", bufs=3) as outp,
        ):
            po_packs = [
                pops.tile([128, min(PACK, NT - i * PACK) * W], f32,
                          name=f"po{i}{sfx}", tag=f"po{i}")
                for i in range((NT + PACK - 1) // PACK)
            ]

            def po_slice(nt):
                return po_packs[nt // PACK][:, (nt % PACK) * W:
                                            (nt % PACK) * W + W]

            hT_bufs = {}

            def emit_phase1(c):
                hT_c = hTp.tile([128, CET * NT * 128], bf16, tag="hTc",
                                name=f"hTc{c}{sfx}")
                hT_bufs[c] = hT_c
                p1a = p1ps.tile([128, 4 * W], f32, tag="p1a",
                                name=f"p1a{c}{sfx}")
                p1b = p1ps.tile([128, 4 * W], f32, tag="p1b",
                                name=f"p1b{c}{sfx}")
                for k in range(NT):
                    hf = hfp.tile([128, CCOLS], f32, tag="hf",
                                  name=f"hf{c}_{k}{sfx}")
                    nc.sync.dma_start(
                        hf[:], H_ext[k * 128:(k + 1) * 128,
                                     c * CCOLS:(c + 1) * CCOLS])
                    hb = hbp.tile([128, CCOLS], bf16, tag="hb",
                                  name=f"hb{c}_{k}{sfx}")
                    if k % 2 == 0:
                        nc.vector.tensor_copy(hb[:], hf[:])
                    else:
                        nc.scalar.activation(
                            hb[:], hf[:], mybir.ActivationFunctionType.Copy)
                    if USE_XBAR:
                        dst = hT_c[:, k * CET * 128:(k + 1) * CET * 128]
                        nc.scalar.dma_start(
                            dst.rearrange("p (j n) -> p j n", n=128),
                            hb[:, :], transpose=True)
                    else:
                        for et in range(CET):
                            ptr = ptps.tile([128, 128], f32, tag="ptr",
                                            name=f"ptr{c}_{k}_{et}{sfx}")
                            nc.tensor.transpose(
                                ptr[:], hb[:, et * 128:(et + 1) * 128],
                                identb[:])
                            eng = nc.vector if et % 2 == 0 else nc.scalar
                            if et % 2 == 0:
                                nc.vector.tensor_copy(
                                    hT_c[:, (k * CET + et) * 128:
                                         (k * CET + et + 1) * 128], ptr[:])
                            else:
                                nc.scalar.activation(
                                    hT_c[:, (k * CET + et) * 128:
                                         (k * CET + et + 1) * 128], ptr[:],
                                    mybir.ActivationFunctionType.Copy)
                    for et in range(CET):
                        ps = p1a if et < 4 else p1b
                        # start=True zeroes the ENTIRE psum bank (measured),
                        # so only the first matmul touching each bank sets it;
                        # the other packed slices accumulate onto the zeroed
                        # bank.
                        nc.tensor.matmul(
                            ps[:, (et % 4) * W:(et % 4) * W + W],
                            hb[:, et * 128:(et + 1) * 128],
                            xp1[:, k * W:(k + 1) * W],
                            start=(k == 0 and et % 4 == 0),
                            stop=(k == NT - 1),
                            skip_group_check=True)
                ar_in = arp.tile([128, CET * W], f32, tag="arin",
                                 name=f"arin{c}{sfx}")
                nc.vector.tensor_copy(ar_in[:, 0:4 * W], p1a[:])
                nc.vector.tensor_copy(ar_in[:, 4 * W:8 * W], p1b[:])
                nc.sync.dma_start(bins[c][:], ar_in[:])
                if SKIP_COLLECTIVE:
                    nc.sync.dma_start(bouts[c][:], bins[c][:])
                else:
                    nc.gpsimd.collective_compute(
                        "AllReduce", mybir.AluOpType.add,
                        replica_groups=[list(range(N_CORES))],
                        ins=[bins[c].opt()], outs=[bouts[c].opt()])

            def emit_phase4(c):
                hT_c = hT_bufs.pop(c)
                eftf = efp.tile([128, CET * W], f32, tag="eftf",
                                name=f"eftf{c}{sfx}")
                nc.sync.dma_start(eftf[:], bouts[c][:])
                eft1 = efp.tile([128, CET * W], bf16, tag="eft1",
                                name=f"eft1{c}{sfx}")
                de = outp.tile([128, CET], f32, tag="de", name=f"de{c}{sfx}")
                nc.vector.tensor_scalar_max(
                    de[:], eftf[:, OUT_CH::W], 1e-20)
                rec = outp.tile([128, CET], f32, tag="rec",
                                name=f"rec{c}{sfx}")
                nc.vector.reciprocal(rec[:], de[:])
                for et in range(CET):
                    nc.vector.tensor_scalar_mul(
                        eft1[:, et * W:et * W + OUT_CH],
                        eftf[:, et * W:et * W + OUT_CH],
                        rec[:, et:et + 1])
                nc.vector.memset(eft1[:, OUT_CH::W], 1.0)
                for nt in range(NT):
                    for et in range(CET):
                        # bank-first matmul only (see phase-1 note): the three
                        # po packs are zeroed by nt 0/7/14's first matmul at
                        # chunk 0; everything else accumulates.
                        nc.tensor.matmul(
                            po_slice(nt),
                            hT_c[:, (nt * CET + et) * 128:
                                 (nt * CET + et + 1) * 128],
                            eft1[:, et * W:(et + 1) * W],
                            start=(c == 0 and et == 0 and nt % PACK == 0),
                            stop=(c == NCH - 1 and et == CET - 1),
                            skip_group_check=True)

            for s in range(NCH + D):
                if s < NCH:
                    emit_phase1(s)
                if s >= D:
                    emit_phase4(s - D)

            # ---- output: out = po[:, 0:64] / max(po[:, 64], eps) ----
            for nt in range(NT):
                po = po_slice(nt)
                dn = outp.tile([128, 1], f32, tag="dn", name=f"dn{nt}{sfx}")
                nc.vector.tensor_scalar_max(dn[:], po[:, OUT_CH:W], 1e-20)
                rcn = outp.tile([128, 1], f32, tag="rcn", name=f"rcn{nt}{sfx}")
                nc.vector.reciprocal(rcn[:], dn[:])
                ot = outp.tile([128, OUT_CH], f32, tag="ot",
                               name=f"ot{nt}{sfx}")
                nc.vector.tensor_scalar_mul(ot[:], po[:, 0:OUT_CH], rcn[:])
                nc.sync.dma_start(out_ext[nt * 128:(nt + 1) * 128, :], ot[:])
                if chain_out is not None and nt == NT - 1:
                    nc.sync.dma_start(chain_out[:], ot[0:1, 0:1])


def build_graph(reps=1, chain=False):
    nc = bacc.Bacc("TRN2", target_bir_lowering=False, debug=False,
                   num_devices=N_CORES)
    x_ext = nc.dram_tensor("x", [NL, IN_CH], f32, kind="ExternalInput")
    H_ext = nc.dram_tensor("H", [NL, EP], f32, kind="ExternalInput")
    th_ext = nc.dram_tensor("theta", [IN_CH, OUT_CH], f32, kind="ExternalInput")
    id_ext = nc.dram_tensor("ident", [128, 128], f32, kind="ExternalInput")
    out_ext = nc.dram_tensor("out", [NL, OUT_CH], f32, kind="ExternalOutput")
    chains = [
        nc.dram_tensor(f"chain{r}", [1, 1], f32, kind="Internal")
        for r in range(reps - 1)
    ] if chain else []
    with tile.TileContext(nc) as tc:
        for r in range(reps):
            ci = chains[r - 1] if (chain and r > 0) else None
            co = chains[r] if (chain and r < reps - 1) else None
            build_body(nc, tc, x_ext, H_ext, th_ext, id_ext, out_ext,
                       sfx=str(r), chain_in=ci, chain_out=co)
    nc.compile()
    return nc


def make_in_maps(x, H, theta):
    x_pad = np.zeros((NL * N_CORES, IN_CH), np.float32)
    x_pad[:N] = x
    H_pad = np.zeros((NL * N_CORES, EP), np.float32)
    H_pad[:N, :E] = H
    ident = np.eye(128, dtype=np.float32)
    theta = np.asarray(theta, np.float32)
    in_maps = []
    for c in range(N_CORES):
        in_maps.append({
            "x": x_pad[c * NL:(c + 1) * NL],
            "H": H_pad[c * NL:(c + 1) * NL],
            "theta": theta,
            "ident": ident,
        })
    return in_maps


def kernel(x, H, theta):
    x = np.asarray(x, np.float32)
    H = np.asarray(H, np.float32)
    nc = build_graph(reps=1)
    in_maps = make_in_maps(x, H, theta)
    res = run_bass_kernel_spmd(nc, in_maps, core_ids=list(range(N_CORES)))
    out = np.concatenate(
        [res.results[c]["out"] for c in range(N_CORES)], axis=0)
    return np.ascontiguousarray(out[:N])


if __name__ == "__main__":
    rng = np.random.default_rng(0)
    x = rng.standard_normal((N, IN_CH), dtype=np.float32)
    H = rng.random((N, E), dtype=np.float32)
    theta = (rng.standard_normal((IN_CH, OUT_CH), dtype=np.float32)
             / np.sqrt(IN_CH))
    out = kernel(x, H, theta)
    xp = x @ theta
    de = H.sum(0)
    eft = (H.T @ xp) / de[:, None]
    dn = H.sum(1)
    ref = (H @ eft) / dn[:, None]
    err = np.abs(out - ref).max() / np.abs(ref).max()
    print("rel err:", err)
